# revision 35
# baseline (speedup 1.0000x reference)
"""Trainium2 Bass kernel for nn_CPCModel (CPC-style NCE loss).

Strategy (8 NeuronCores, full inputs on every core, no collectives):

The reference's leave-one-out softmax pooling collapses algebraically:
    pooled[i] = (T - e_i * zt_i) / (S - e_i),  e = exp(s), S = sum(e), T = sum(e_j zt_j)
so the [B,B] pooling matrix is never materialized.  The loss needs only
    nce = -mean_i( total[i,i] - logsumexp_j total[i,j] )
with  total[i, j in group g] = Azw_g[i]·pooled_g[j] + Czw[i]·c[j] + delta_g[i]
where Azw_g = zw @ Ww_g, Czw = zw @ Wk_w, delta_g = zw @ (Ww_g_b + Wk_b).

Each core redundantly computes the cheap pooling prep for all 4096 rows
(both groups stacked on the 128 partitions) and its own 512 rows of the
[4096,4096] total matrix + row-wise sum(exp(total - 44)) via 8
[128,2048] exp chunks on the scalar engine with accum_out.  The loop
rhs V2 = [cT ; pooled0|pooled1] is assembled by three DMAs (no compute)
and doubles as the diagonal's source through one partition_id-dynamic
column slice.  Raw per-row exp sums and raw diagonal values ship to the
host, which finishes with log() in float64.

Scheduling: weights arrive in a small first DMA and zw^T in two
chunk-major halves so the zt->h->s chain starts ~4us in; exp-of-s is
chunked so the fused ztw/T tensor_tensor_reduce starts before the last
tanh; U/bias matmuls and paced PE probes keep the tensor engine out of
its low p-state before the main loop.  fp32r matmuls keep K=128
throughout (K=64 fp32r dies at runtime).  One activation-table set
(exp_and_others: relu/tanh/exp/copy) serves the whole kernel; the final
log runs on the host.
"""

import numpy as np

import concourse.bacc as bacc
import concourse.bass as bass
import concourse.mybir as mybir
import concourse.tile as tile
from concourse.bass_utils import run_bass_kernel_spmd

N_CORES = 8
B = 4096
OWN = B // N_CORES            # 512 rows of `total` per core
G = 2048                      # group size
F32 = mybir.dt.float32
F32R = mybir.dt.float32r
BF16 = mybir.dt.bfloat16
AF = mybir.ActivationFunctionType
ALU = mybir.AluOpType
SHIFT = 44.0

# bf16 weights bundle column offsets
ZWO = 0           # zwoTb [128, 512]
LW0 = 512         # lin0_w.T [128, 64]
LW1 = 576         # lin1_w.T [128, 64]
A1W = 640         # blockdiag(a0_1w.T, a1_1w.T) [128, 64]
A2W = 704         # blockdiag(a0_2w.T, a1_2w.T) [64, 2]
SEL = 706         # sel2 [2, 128]
BSTK = 834        # [b0 | b1] [128, 2]
BOWN = 836        # b_{group(core)} [128, 1]
BONE = 837        # ones [128, 1]
WN = 838

# f32r bundle column offsets
ZWOF = 0          # zwoT [128, 512]
CU0 = 512         # hstack(Wk_w, Ww0_w) [128, 128]
CU1 = 640         # hstack(Wk_w, Ww1_w)
CUWO = 768        # per-core hstack(Wk_w, Ww_g) [128, 128] (diag)
NR = 896
# f32 bundle column offsets
LINB = 0          # [lin0_b ; lin1_b] [128, 1]
A1B = 1           # [a0_1b ; a1_1b] [64, 1]
A2B = 2           # [a0_2b ; a1_2b] [2, 1]
NF32 = 3


def _build_program(static_diag=False):
    nc = bacc.Bacc(
        "TRN2",
        target_bir_lowering=False,
        debug=False,
        num_devices=N_CORES,
    )

    bigw_d = nc.dram_tensor("bigw", [128, WN], BF16, kind="ExternalInput").ap()
    zwc_d = nc.dram_tensor("zwc", [128, B], BF16, kind="ExternalInput").ap()
    bigr_d = nc.dram_tensor("bigr", [128, NR], F32R, kind="ExternalInput").ap()
    bigf_d = nc.dram_tensor("bigf", [128, NF32], F32, kind="ExternalInput").ap()
    cT_d = nc.dram_tensor("cT", [64, B], F32R, kind="ExternalInput").ap()
    vout_d = nc.dram_tensor("vout", [128, 20], F32, kind="ExternalOutput").ap()

    from contextlib import ExitStack
    with tile.TileContext(nc) as tc, ExitStack() as ctx:
        pers = ctx.enter_context(tc.tile_pool(name="pers", bufs=1))
        scr = ctx.enter_context(tc.tile_pool(name="scr", bufs=2))

        # DMA order = need order: weights, zw halves, f32r/f32 bundles, cT
        bw = pers.tile([128, WN], BF16, tag="bw", name="bw")
        nc.sync.dma_start(bw[:], bigw_d[:])
        bf = pers.tile([128, NF32], F32, tag="bf", name="bf")
        nc.sync.dma_start(bf[:], bigf_d[:])
        zwc = pers.tile([128, B], BF16, tag="zwc", name="zwc")
        for p in range(4):
            nc.sync.dma_start(zwc[:, p * 1024:(p + 1) * 1024],
                              zwc_d[:, p * 1024:(p + 1) * 1024])
        br = pers.tile([128, NR], F32R, tag="br", name="br")
        nc.sync.dma_start(br[:], bigr_d[:])
        # V2 = [cT ; pooled0|pooled1]: loop rhs AND diagonal source
        V2 = pers.tile([128, B], F32R, tag="V2", name="V2")
        nc.sync.dma_start(V2[0:64, :], cT_d[:])

        zwoT = br[:, ZWOF:ZWOF + 512]
        sel2 = bw[0:2, SEL:SEL + 128]
        linb2 = bf[:, LINB:LINB + 1]
        a1b2 = bf[0:64, A1B:A1B + 1]
        a2b2 = bf[0:2, A2B:A2B + 1]

        ztT2 = pers.tile([128, G], BF16, tag="ztT2")
        hT2 = pers.tile([64, G], BF16, tag="hT2")
        eT2 = pers.tile([2, G], BF16, tag="eT2")
        d2 = pers.tile([2, G], BF16, tag="d2")
        bT2 = pers.tile([2, G], BF16, tag="bT2")
        ztwT2 = pers.tile([128, G], F32, tag="ztwT2")
        pooled2 = pers.tile([128, G], F32R, tag="pooled2")
        Sacc = pers.tile([2, 4], F32, tag="Sacc")
        Tacc = pers.tile([128, 4], F32, tag="Tacc")
        T2 = pers.tile([128, 1], F32, tag="T2")
        S2 = pers.tile([2, 1], F32, tag="S2")
        biasS = pers.tile([128, 8], F32, tag="biasS")
        U0 = pers.tile([128, OWN], F32R, tag="U0")
        U1 = pers.tile([128, OWN], F32R, tag="U1")
        UOwnS = pers.tile([128, OWN], F32, tag="UOwnS")
        vout = pers.tile([128, 20], F32, tag="vout")

        actwarm = pers.tile([2, 1], BF16, tag="actwarm")

        with tc.tile_pool(name="psA", bufs=6, space="PSUM") as psA, \
             tc.tile_pool(name="psB", bufs=1, space="PSUM") as psB:
            psU = psA
            # trigger the one-time activation table load before tanh needs it
            nc.scalar.activation(actwarm[:], bw[0:2, 0:1], AF.Tanh)
            # two PE warmups bridge the gap until zw chunks arrive
            for _ in range(2):
                pw0 = psA.tile([128, 512], F32, tag="ps")
                nc.tensor.matmul(pw0[:], bw[:, 0:128], bw[:, 0:512],
                                 start=True, stop=True)

            # ---- zt -> h -> s -> e chain, issued breadth-first so each
            # engine's in-order queue never blocks the next chunk ----
            SL = [slice(ch * 512, (ch + 1) * 512) for ch in range(4)]
            pzs = []
            for ch in range(4):
                pz = psA.tile([128, 512], F32, tag="ps")
                nc.tensor.matmul(pz[0:64, :], bw[:, LW0:LW0 + 64],
                                 zwc[:, ch * 1024:ch * 1024 + 512],
                                 start=True, stop=True)
                nc.tensor.matmul(pz[64:128, :], bw[:, LW1:LW1 + 64],
                                 zwc[:, ch * 1024 + 512:(ch + 1) * 1024],
                                 start=True, stop=True)
                pzs.append(pz)
            for ch in range(4):
                # relu(x + bias): split across Act and DVE
                if ch % 2 == 0:
                    nc.scalar.activation(ztT2[:, SL[ch]], pzs[ch][:], AF.Relu,
                                         bias=linb2)
                else:
                    nc.vector.tensor_scalar(ztT2[:, SL[ch]], pzs[ch][:], linb2,
                                            0.0, op0=ALU.add, op1=ALU.max)
            phs = []
            for ch in range(4):
                ph = psA.tile([128, 512], F32, tag="ps")
                nc.tensor.matmul(ph[0:64, :], bw[:, A1W:A1W + 64],
                                 ztT2[:, SL[ch]], start=True, stop=True)
                phs.append(ph)
            pss = []
            for ch in range(4):
                ps_ = psA.tile([128, 512], F32, tag="ps")
                pss.append(ps_)
            for ch in range(4):
                nc.scalar.activation(hT2[:, SL[ch]], phs[ch][0:64, :], AF.Tanh,
                                     bias=a1b2)
                nc.tensor.matmul(pss[ch][0:2, :], bw[0:64, A2W:A2W + 2],
                                 hT2[:, SL[ch]], start=True, stop=True)
                nc.scalar.activation(eT2[:, SL[ch]], pss[ch][0:2, :], AF.Exp,
                                     bias=a2b2, accum_out=Sacc[:, ch:ch + 1])

            # ---- e-broadcast, ztw on DVE, T-partials on Act (accum) ----
            pebs = []
            for ch in range(4):
                peb = psA.tile([128, 512], F32, tag="ps")
                nc.tensor.matmul(peb[:], sel2, eT2[:, SL[ch]],
                                 start=True, stop=True)
                pebs.append(peb)
            for ch in range(4):
                nc.vector.tensor_tensor(ztwT2[:, SL[ch]], ztT2[:, SL[ch]],
                                        pebs[ch][:], op=ALU.mult)
                nc.vector.reduce_sum(Tacc[:, ch:ch + 1], ztwT2[:, SL[ch]],
                                     axis=mybir.AxisListType.X)

            # ---- beta = 1/(e - S) on DVE ----
            nc.vector.reduce_sum(S2[:], Sacc[:], axis=mybir.AxisListType.X)
            nc.vector.tensor_scalar(d2[:], eT2[:], S2[:], None,
                                    op0=ALU.subtract)

            # ---- U tiles + delta biases (PE slack while beta computes) ----
            upus = []
            for uoff in (CU0, CU1):
                pu = psA.tile([128, 512], F32, tag="ps")
                nc.tensor.matmul(pu[:], br[:, uoff:uoff + 128],
                                 zwoT, start=True, stop=True)
                upus.append(pu)
            puo = psA.tile([128, 512], F32, tag="ps")
            nc.tensor.matmul(puo[:], br[:, CUWO:CUWO + 128],
                             zwoT, start=True, stop=True)
            nc.scalar.copy(UOwnS[:], puo[:])
            pbias = psB.tile([128, 8], F32, tag="pb")
            for ic in range(4):
                nc.tensor.matmul(pbias[:, ic * 2:ic * 2 + 2],
                                 bw[:, ZWO + ic * 128:ZWO + (ic + 1) * 128],
                                 bw[:, BSTK:BSTK + 2], start=True, stop=True)
            nc.vector.tensor_scalar(biasS[:], pbias[:], -SHIFT, None,
                                    op0=ALU.add)

            # ---- beta chunks, T2, pooled combines: interleaved on DVE.
            # Group-1 combines write straight into V2 (partition-aligned), so
            # the loop's first exp chunks start without any DMA hop; group-0
            # combines land in pooled2 and shift to V2 via DMA during the
            # group-1 half of the loop.
            def recip(ch):
                with nc.allow_low_precision(reason="beta in bf16 for PE bcast"):
                    nc.vector.reciprocal(bT2[:, SL[ch]], d2[:, SL[ch]])

            def bcast(ch):
                pbb = psA.tile([128, 512], F32, tag="ps", name=f"pbb{ch}")
                nc.tensor.matmul(pbb[:], sel2, bT2[:, SL[ch]],
                                 start=True, stop=True)
                return pbb

            def combine(ch, pbb):
                nc.vector.scalar_tensor_tensor(
                    out=pooled2[:, SL[ch]], in0=ztwT2[:, SL[ch]], scalar=T2[:],
                    in1=pbb[:], op0=ALU.subtract, op1=ALU.mult)

            recip(0)
            pbb0 = bcast(0)
            recip(1)
            pbb1 = bcast(1)
            nc.vector.reduce_sum(T2[:], Tacc[:], axis=mybir.AxisListType.X)
            combine(0, pbb0)
            # group-1 V2 pieces stream per-chunk: the loop runs group 1 first
            nc.sync.dma_start(V2[64:128, G:G + 512], pooled2[64:128, 0:512])
            recip(2)
            pbb2 = bcast(2)
            combine(1, pbb1)
            nc.sync.dma_start(V2[64:128, G + 512:G + 1024],
                              pooled2[64:128, 512:1024])
            recip(3)
            pbb3 = bcast(3)
            combine(2, pbb2)
            nc.sync.dma_start(V2[64:128, G + 1024:G + 1536],
                              pooled2[64:128, 1024:1536])
            combine(3, pbb3)
            nc.sync.dma_start(V2[64:128, G + 1536:B], pooled2[64:128, 1536:G])
            nc.sync.dma_start(V2[64:128, 0:1024], pooled2[0:64, 0:1024])
            nc.sync.dma_start(V2[64:128, 1024:G], pooled2[0:64, 1024:G])
            # paced junk matmuls: keep the PE out of its low p-state between
            # the prep matmuls and the main loop
            for ch in range(4):
                pw = psA.tile([128, 512], F32, tag="ps")
                nc.tensor.matmul(pw[:], br[:, CU0:CU0 + 128],
                                 pooled2[:, SL[ch]], start=True, stop=True)

            # U copies on Act: needed only once the loop starts
            nc.scalar.copy(U1[:], upus[1][:])
            nc.scalar.copy(U0[:], upus[0][:])

        # ---- main loop over the [512 own rows, 4096 cols] of `total`:
        # mostly 2048-col exp chunks; the first two group-1 chunks split in
        # 1024-col halves so exp work starts as soon as the first half of
        # pooled1 lands in V2 (the rest of the tail still streaming) ----
        with tc.tile_pool(name="pbig", bufs=2, space="PSUM") as pbig:
            def loop_chunk(g, ic, h, width, cc):
                usl = slice(ic * 128, (ic + 1) * 128)
                Ug = U0 if g == 0 else U1
                base = g * G + h * 1024
                pm = pbig.tile([128, width], F32, tag="pb",
                               padded_shape=[128, G])
                for q in range(width // 512):
                    qs = slice(q * 512, (q + 1) * 512)
                    nc.tensor.matmul(pm[:, qs], Ug[:, usl],
                                     V2[:, base + q * 512:
                                        base + (q + 1) * 512],
                                     start=True, stop=True)
                es = scr.tile([128, width], BF16, tag="es",
                              padded_shape=[128, G])
                nc.scalar.activation(es[:], pm[:], AF.Exp,
                                     bias=biasS[:, 2 * ic + g:2 * ic + g + 1],
                                     accum_out=vout[:, cc:cc + 1])

            loop_chunk(1, 0, 0, 1024, 12)
            loop_chunk(1, 0, 1, 1024, 13)
            loop_chunk(1, 1, 0, 1024, 14)
            loop_chunk(1, 1, 1, 1024, 15)
            for g, ic in [(1, 2), (1, 3), (0, 0), (0, 1), (0, 2), (0, 3)]:
                loop_chunk(g, ic, 0, G, g * 4 + ic)

        # ---- diagonal: diag[i] = UOwn[:,i]·V2[:,own(i)] + delta ----
        with tc.tile_pool(name="ptail", bufs=1, space="PSUM") as pt:
            if static_diag:
                csl = slice(0, OWN)
            else:
                pid = nc.vector.partition_id()
                csl = bass.ts(pid, OWN)
            prod = pers.tile([128, OWN], BF16, tag="prod")
            nc.vector.tensor_tensor(prod[0:64, :], UOwnS[0:64, :],
                                    V2[0:64, csl].bitcast(F32), op=ALU.mult)
            nc.vector.tensor_tensor(prod[64:128, :], UOwnS[64:128, :],
                                    V2[64:128, csl].bitcast(F32),
                                    op=ALU.mult)
            pdg = pt.tile([128, 4], F32, tag="pt")
            for ic in range(4):
                nc.tensor.matmul(pdg[:, ic:ic + 1],
                                 prod[:, ic * 128:(ic + 1) * 128],
                                 bw[:, BONE:BONE + 1], start=True, stop=False)
                nc.tensor.matmul(pdg[:, ic:ic + 1],
                                 bw[:, ZWO + ic * 128:ZWO + (ic + 1) * 128],
                                 bw[:, BOWN:BOWN + 1], start=False, stop=True)
            nc.vector.tensor_copy(vout[:, 16:20], pdg[:])
            nc.sync.dma_start(vout_d[:, 4:20], vout[:, 4:20])
            nc.sync.dma_start(vout_d[:, 0:4], vout[:, 0:4])

    nc.compile()
    return nc


_built = None


def _get_program():
    global _built
    if _built is None:
        _built = _build_program()
    return _built


def make_in_maps(inputs):
    import ml_dtypes
    BF = ml_dtypes.bfloat16
    f = lambda x: np.asarray(x, dtype=np.float32)

    zw = np.concatenate([f(inputs['zw_0']), f(inputs['zw_1'])], axis=0)
    zwT = np.ascontiguousarray(zw.T)                  # [128, 4096]
    # chunk-major layout: block ch = [group0 cols ch*512.. | group1 cols ...]
    zwc = np.empty_like(zwT)
    for ch in range(4):
        zwc[:, ch * 1024:ch * 1024 + 512] = zwT[:, ch * 512:(ch + 1) * 512]
        zwc[:, ch * 1024 + 512:(ch + 1) * 1024] = \
            zwT[:, G + ch * 512:G + (ch + 1) * 512]
    b0 = f(inputs['Ww0_b']) + f(inputs['Wk_b'])
    b1 = f(inputs['Ww1_b']) + f(inputs['Wk_b'])
    wk = f(inputs['Wk_w'])
    uw0 = np.hstack([wk, f(inputs['Ww0_w'])])          # [Czw ; Azw0]
    uw1 = np.hstack([wk, f(inputs['Ww1_w'])])          # [Czw ; Azw1]

    bigw = np.zeros((128, WN), np.float32)
    bigw[:, LW0:LW0 + 64] = f(inputs['lin0_w']).T
    bigw[:, LW1:LW1 + 64] = f(inputs['lin1_w']).T
    bigw[0:64, A1W:A1W + 32] = f(inputs['a0_1w']).T
    bigw[64:128, A1W + 32:A1W + 64] = f(inputs['a1_1w']).T
    bigw[0:32, A2W:A2W + 1] = f(inputs['a0_2w']).T
    bigw[32:64, A2W + 1:A2W + 2] = f(inputs['a1_2w']).T
    bigw[0, SEL:SEL + 64] = 1.0
    bigw[1, SEL + 64:SEL + 128] = 1.0
    bigw[:, BSTK] = b0
    bigw[:, BSTK + 1] = b1
    bigw[:, BONE] = 1.0

    bigr = np.zeros((128, NR), np.float32)
    bigr[:, CU0:CU0 + 128] = uw0
    bigr[:, CU1:CU1 + 128] = uw1
    bigf = np.zeros((128, NF32), np.float32)
    bigf[:, LINB] = np.concatenate([f(inputs['lin0_b']), f(inputs['lin1_b'])])
    bigf[0:64, A1B] = np.concatenate([f(inputs['a0_1b']), f(inputs['a1_1b'])])
    bigf[0:2, A2B] = np.concatenate([f(inputs['a0_2b']), f(inputs['a1_2b'])])

    cT = np.ascontiguousarray(f(inputs['c']).T)        # [64, 4096]

    in_maps = []
    for cid in range(N_CORES):
        g = cid // 4
        mw = bigw.copy()
        mw[:, ZWO:ZWO + OWN] = zwT[:, cid * OWN:(cid + 1) * OWN]
        mw[:, BOWN] = b0 if g == 0 else b1
        mr = bigr.copy()
        mr[:, ZWOF:ZWOF + OWN] = zwT[:, cid * OWN:(cid + 1) * OWN]
        mr[:, CUWO:CUWO + 128] = uw0 if g == 0 else uw1
        in_maps.append({
            'bigw': np.ascontiguousarray(mw.astype(BF)),
            'zwc': np.ascontiguousarray(zwc.astype(BF)),
            'bigr': np.ascontiguousarray(mr),
            'bigf': bigf,
            'cT': cT,
        })
    return in_maps


def kernel(**inputs):
    nc = _get_program()
    in_maps = make_in_maps(inputs)
    res = run_bass_kernel_spmd(nc, in_maps, list(range(N_CORES)))
    tot = 0.0
    for r in res.results:
        v = np.asarray(r['vout'], dtype=np.float64)
        v[:, 4] = v[:, 12] + v[:, 13]       # split chunks (g1, ic0/ic1)
        v[:, 5] = v[:, 14] + v[:, 15]
        se = v[:, 0:4] + v[:, 4:8]          # [128, 4]: sum over both groups
        dg = v[:, 8:12]
        tot += np.sum(dg - SHIFT - np.log(se))
    return np.array(-(tot / B), dtype=np.float32)


# revision 36
# speedup vs baseline: 1.0157x; 1.0157x over previous
"""Trainium2 Bass kernel for nn_CPCModel (CPC-style NCE loss).

Strategy (8 NeuronCores, full inputs on every core, no collectives):

The reference's leave-one-out softmax pooling collapses algebraically:
    pooled[i] = (T - e_i * zt_i) / (S - e_i),  e = exp(s), S = sum(e), T = sum(e_j zt_j)
so the [B,B] pooling matrix is never materialized.  The loss needs only
    nce = -mean_i( total[i,i] - logsumexp_j total[i,j] )
with  total[i, j in group g] = Azw_g[i]·pooled_g[j] + Czw[i]·c[j] + delta_g[i]
where Azw_g = zw @ Ww_g, Czw = zw @ Wk_w, delta_g = zw @ (Ww_g_b + Wk_b).

Each core redundantly computes the cheap pooling prep for all 4096 rows
(both groups stacked on the 128 partitions) and its own 512 rows of the
[4096,4096] total matrix + row-wise sum(exp(total - 44)) via 8
[128,2048] exp chunks on the scalar engine with accum_out.  The loop
rhs V2 = [cT ; pooled0|pooled1] is assembled by three DMAs (no compute)
and doubles as the diagonal's source through one partition_id-dynamic
column slice.  Raw per-row exp sums and raw diagonal values ship to the
host, which finishes with log() in float64.

Scheduling: weights arrive in a small first DMA and zw^T in two
chunk-major halves so the zt->h->s chain starts ~4us in; exp-of-s is
chunked so the fused ztw/T tensor_tensor_reduce starts before the last
tanh; U/bias matmuls and paced PE probes keep the tensor engine out of
its low p-state before the main loop.  fp32r matmuls keep K=128
throughout (K=64 fp32r dies at runtime).  One activation-table set
(exp_and_others: relu/tanh/exp/copy) serves the whole kernel; the final
log runs on the host.
"""

import numpy as np

import concourse.bacc as bacc
import concourse.bass as bass
import concourse.mybir as mybir
import concourse.tile as tile
from concourse.bass_utils import run_bass_kernel_spmd

N_CORES = 8
B = 4096
OWN = B // N_CORES            # 512 rows of `total` per core
G = 2048                      # group size
F32 = mybir.dt.float32
F32R = mybir.dt.float32r
BF16 = mybir.dt.bfloat16
AF = mybir.ActivationFunctionType
ALU = mybir.AluOpType
SHIFT = 44.0

# bf16 weights bundle column offsets
ZWO = 0           # zwoTb [128, 512]
LW0 = 512         # lin0_w.T [128, 64]
LW1 = 576         # lin1_w.T [128, 64]
A1W = 640         # blockdiag(a0_1w.T, a1_1w.T) [128, 64]
A2W = 704         # blockdiag(a0_2w.T, a1_2w.T) [64, 2]
SEL = 706         # sel2 [2, 128]
BSTK = 834        # [b0 | b1] [128, 2]
BOWN = 836        # b_{group(core)} [128, 1]
BONE = 837        # ones [128, 1]
WN = 838

# f32r bundle column offsets
ZWOF = 0          # zwoT [128, 512]
CU0 = 512         # hstack(Wk_w, Ww0_w) [128, 128]
CU1 = 640         # hstack(Wk_w, Ww1_w)
CUWO = 768        # per-core hstack(Wk_w, Ww_g) [128, 128] (diag)
NR = 896
# f32 bundle column offsets
LINB = 0          # [lin0_b ; lin1_b] [128, 1]
A1B = 1           # [a0_1b ; a1_1b] [64, 1]
A2B = 2           # [a0_2b ; a1_2b] [2, 1]
NF32 = 3


def _build_program(static_diag=False):
    nc = bacc.Bacc(
        "TRN2",
        target_bir_lowering=False,
        debug=False,
        num_devices=N_CORES,
    )

    bigw_d = nc.dram_tensor("bigw", [128, WN], BF16, kind="ExternalInput").ap()
    zwc_d = nc.dram_tensor("zwc", [128, B], BF16, kind="ExternalInput").ap()
    bigr_d = nc.dram_tensor("bigr", [128, NR], F32R, kind="ExternalInput").ap()
    bigf_d = nc.dram_tensor("bigf", [128, NF32], F32, kind="ExternalInput").ap()
    cT_d = nc.dram_tensor("cT", [64, B], F32R, kind="ExternalInput").ap()
    vout_d = nc.dram_tensor("vout", [128, 20], F32, kind="ExternalOutput").ap()

    from contextlib import ExitStack
    with tile.TileContext(nc) as tc, ExitStack() as ctx:
        pers = ctx.enter_context(tc.tile_pool(name="pers", bufs=1))
        scr = ctx.enter_context(tc.tile_pool(name="scr", bufs=2))

        # DMA order = need order: weights, zw halves, f32r/f32 bundles, cT
        bw = pers.tile([128, WN], BF16, tag="bw", name="bw")
        nc.sync.dma_start(bw[:], bigw_d[:])
        bf = pers.tile([128, NF32], F32, tag="bf", name="bf")
        nc.sync.dma_start(bf[:], bigf_d[:])
        zwc = pers.tile([128, B], BF16, tag="zwc", name="zwc")
        for p in range(4):
            nc.sync.dma_start(zwc[:, p * 1024:(p + 1) * 1024],
                              zwc_d[:, p * 1024:(p + 1) * 1024])
        br = pers.tile([128, NR], F32R, tag="br", name="br")
        nc.sync.dma_start(br[:], bigr_d[:])
        # V2 = [cT ; pooled0|pooled1]: loop rhs AND diagonal source
        V2 = pers.tile([128, B], F32R, tag="V2", name="V2")
        nc.sync.dma_start(V2[0:64, :], cT_d[:])

        zwoT = br[:, ZWOF:ZWOF + 512]
        sel2 = bw[0:2, SEL:SEL + 128]
        linb2 = bf[:, LINB:LINB + 1]
        a1b2 = bf[0:64, A1B:A1B + 1]
        a2b2 = bf[0:2, A2B:A2B + 1]

        ztT2 = pers.tile([128, G], BF16, tag="ztT2")
        hT2 = pers.tile([64, G], BF16, tag="hT2")
        eT2 = pers.tile([2, G], BF16, tag="eT2")
        d2 = pers.tile([2, G], BF16, tag="d2")
        bT2 = pers.tile([2, G], BF16, tag="bT2")
        ztwT2 = pers.tile([128, G], F32, tag="ztwT2")
        pooled2 = pers.tile([128, G], F32R, tag="pooled2")
        Sacc = pers.tile([2, 4], F32, tag="Sacc")
        Tacc = pers.tile([128, 4], F32, tag="Tacc")
        T2 = pers.tile([128, 1], F32, tag="T2")
        S2 = pers.tile([2, 1], F32, tag="S2")
        biasS = pers.tile([128, 8], F32, tag="biasS")
        U0 = pers.tile([128, OWN], F32R, tag="U0")
        U1 = pers.tile([128, OWN], F32R, tag="U1")
        UOwnS = pers.tile([128, OWN], F32, tag="UOwnS")
        vout = pers.tile([128, 20], F32, tag="vout")

        actwarm = pers.tile([2, 1], BF16, tag="actwarm")

        with tc.tile_pool(name="psA", bufs=6, space="PSUM") as psA, \
             tc.tile_pool(name="psB", bufs=1, space="PSUM") as psB:
            psU = psA
            # trigger the one-time activation table load before tanh needs it
            nc.scalar.activation(actwarm[:], bw[0:2, 0:1], AF.Tanh)
            # two PE warmups bridge the gap until zw chunks arrive
            for _ in range(2):
                pw0 = psA.tile([128, 512], F32, tag="ps")
                nc.tensor.matmul(pw0[:], bw[:, 0:128], bw[:, 0:512],
                                 start=True, stop=True)

            # ---- zt -> h -> s -> e chain, issued breadth-first so each
            # engine's in-order queue never blocks the next chunk ----
            SL = [slice(ch * 512, (ch + 1) * 512) for ch in range(4)]
            pzs = []
            for ch in range(4):
                pz = psA.tile([128, 512], F32, tag="ps")
                nc.tensor.matmul(pz[0:64, :], bw[:, LW0:LW0 + 64],
                                 zwc[:, ch * 1024:ch * 1024 + 512],
                                 start=True, stop=True)
                nc.tensor.matmul(pz[64:128, :], bw[:, LW1:LW1 + 64],
                                 zwc[:, ch * 1024 + 512:(ch + 1) * 1024],
                                 start=True, stop=True)
                pzs.append(pz)
            for ch in range(4):
                # relu(x + bias): split across Act and DVE
                if ch % 2 == 0:
                    nc.scalar.activation(ztT2[:, SL[ch]], pzs[ch][:], AF.Relu,
                                         bias=linb2)
                else:
                    nc.vector.tensor_scalar(ztT2[:, SL[ch]], pzs[ch][:], linb2,
                                            0.0, op0=ALU.add, op1=ALU.max)
            phs = []
            for ch in range(4):
                ph = psA.tile([128, 512], F32, tag="ps")
                nc.tensor.matmul(ph[0:64, :], bw[:, A1W:A1W + 64],
                                 ztT2[:, SL[ch]], start=True, stop=True)
                phs.append(ph)
            pss = []
            for ch in range(4):
                ps_ = psA.tile([128, 512], F32, tag="ps")
                pss.append(ps_)
            for ch in range(4):
                nc.scalar.activation(hT2[:, SL[ch]], phs[ch][0:64, :], AF.Tanh,
                                     bias=a1b2)
                nc.tensor.matmul(pss[ch][0:2, :], bw[0:64, A2W:A2W + 2],
                                 hT2[:, SL[ch]], start=True, stop=True)
                nc.scalar.activation(eT2[:, SL[ch]], pss[ch][0:2, :], AF.Exp,
                                     bias=a2b2, accum_out=Sacc[:, ch:ch + 1])

            # ---- e-broadcast, ztw on DVE, T-partials on Act (accum) ----
            pebs = []
            for ch in range(4):
                peb = psA.tile([128, 512], F32, tag="ps")
                nc.tensor.matmul(peb[:], sel2, eT2[:, SL[ch]],
                                 start=True, stop=True)
                pebs.append(peb)
            for ch in range(4):
                nc.vector.tensor_tensor(ztwT2[:, SL[ch]], ztT2[:, SL[ch]],
                                        pebs[ch][:], op=ALU.mult)
                if ch % 2 == 0:
                    tjunk = scr.tile([128, 512], BF16, tag="tj")
                    nc.scalar.activation(tjunk[:], ztwT2[:, SL[ch]], AF.Copy,
                                         accum_out=Tacc[:, ch:ch + 1])
                else:
                    nc.vector.reduce_sum(Tacc[:, ch:ch + 1], ztwT2[:, SL[ch]],
                                         axis=mybir.AxisListType.X)

            # ---- beta = 1/(e - S) on DVE ----
            nc.vector.reduce_sum(S2[:], Sacc[:], axis=mybir.AxisListType.X)
            nc.vector.tensor_scalar(d2[:], eT2[:], S2[:], None,
                                    op0=ALU.subtract)

            # ---- U tiles + delta biases (PE slack while beta computes) ----
            upus = []
            for uoff in (CU0, CU1):
                pu = psA.tile([128, 512], F32, tag="ps")
                nc.tensor.matmul(pu[:], br[:, uoff:uoff + 128],
                                 zwoT, start=True, stop=True)
                upus.append(pu)
            puo = psA.tile([128, 512], F32, tag="ps")
            nc.tensor.matmul(puo[:], br[:, CUWO:CUWO + 128],
                             zwoT, start=True, stop=True)
            nc.scalar.copy(UOwnS[:], puo[:])
            pbias = psB.tile([128, 8], F32, tag="pb")
            for ic in range(4):
                nc.tensor.matmul(pbias[:, ic * 2:ic * 2 + 2],
                                 bw[:, ZWO + ic * 128:ZWO + (ic + 1) * 128],
                                 bw[:, BSTK:BSTK + 2], start=True, stop=True)
            nc.vector.tensor_scalar(biasS[:], pbias[:], -SHIFT, None,
                                    op0=ALU.add)

            # ---- beta chunks, T2, pooled combines: interleaved on DVE.
            # Group-1 combines write straight into V2 (partition-aligned), so
            # the loop's first exp chunks start without any DMA hop; group-0
            # combines land in pooled2 and shift to V2 via DMA during the
            # group-1 half of the loop.
            def recip(ch):
                with nc.allow_low_precision(reason="beta in bf16 for PE bcast"):
                    nc.vector.reciprocal(bT2[:, SL[ch]], d2[:, SL[ch]])

            def bcast(ch):
                pbb = psA.tile([128, 512], F32, tag="ps", name=f"pbb{ch}")
                nc.tensor.matmul(pbb[:], sel2, bT2[:, SL[ch]],
                                 start=True, stop=True)
                return pbb

            def combine(ch, pbb):
                nc.vector.scalar_tensor_tensor(
                    out=pooled2[:, SL[ch]], in0=ztwT2[:, SL[ch]], scalar=T2[:],
                    in1=pbb[:], op0=ALU.subtract, op1=ALU.mult)

            recip(0)
            pbb0 = bcast(0)
            recip(1)
            pbb1 = bcast(1)
            nc.vector.reduce_sum(T2[:], Tacc[:], axis=mybir.AxisListType.X)
            combine(0, pbb0)
            # group-1 V2 pieces stream per-chunk: the loop runs group 1 first
            nc.sync.dma_start(V2[64:128, G:G + 512], pooled2[64:128, 0:512])
            recip(2)
            pbb2 = bcast(2)
            combine(1, pbb1)
            nc.sync.dma_start(V2[64:128, G + 512:G + 1024],
                              pooled2[64:128, 512:1024])
            recip(3)
            pbb3 = bcast(3)
            combine(2, pbb2)
            nc.sync.dma_start(V2[64:128, G + 1024:G + 1536],
                              pooled2[64:128, 1024:1536])
            combine(3, pbb3)
            nc.sync.dma_start(V2[64:128, G + 1536:B], pooled2[64:128, 1536:G])
            nc.sync.dma_start(V2[64:128, 0:1024], pooled2[0:64, 0:1024])
            nc.sync.dma_start(V2[64:128, 1024:G], pooled2[0:64, 1024:G])
            # paced junk matmuls: keep the PE out of its low p-state between
            # the prep matmuls and the main loop
            for ch in range(4):
                pw = psA.tile([128, 512], F32, tag="ps")
                nc.tensor.matmul(pw[:], br[:, CU0:CU0 + 128],
                                 pooled2[:, SL[ch]], start=True, stop=True)

            # U copies on Act: needed only once the loop starts
            nc.scalar.copy(U1[:], upus[1][:])
            nc.scalar.copy(U0[:], upus[0][:])

        # ---- main loop over the [512 own rows, 4096 cols] of `total`:
        # mostly 2048-col exp chunks; the first two group-1 chunks split in
        # 1024-col halves so exp work starts as soon as the first half of
        # pooled1 lands in V2 (the rest of the tail still streaming) ----
        with tc.tile_pool(name="pbig", bufs=2, space="PSUM") as pbig:
            def loop_chunk(g, ic, h, width, cc):
                usl = slice(ic * 128, (ic + 1) * 128)
                Ug = U0 if g == 0 else U1
                base = g * G + h * 1024
                pm = pbig.tile([128, width], F32, tag="pb",
                               padded_shape=[128, G])
                for q in range(width // 512):
                    qs = slice(q * 512, (q + 1) * 512)
                    nc.tensor.matmul(pm[:, qs], Ug[:, usl],
                                     V2[:, base + q * 512:
                                        base + (q + 1) * 512],
                                     start=True, stop=True)
                es = scr.tile([128, width], BF16, tag="es",
                              padded_shape=[128, G])
                nc.scalar.activation(es[:], pm[:], AF.Exp,
                                     bias=biasS[:, 2 * ic + g:2 * ic + g + 1],
                                     accum_out=vout[:, cc:cc + 1])

            loop_chunk(1, 0, 0, 1024, 12)
            loop_chunk(1, 0, 1, 1024, 13)
            loop_chunk(1, 1, 0, 1024, 14)
            loop_chunk(1, 1, 1, 1024, 15)
            for g, ic in [(1, 2), (1, 3), (0, 0), (0, 1), (0, 2), (0, 3)]:
                loop_chunk(g, ic, 0, G, g * 4 + ic)

        # ---- diagonal: diag[i] = UOwn[:,i]·V2[:,own(i)] + delta ----
        with tc.tile_pool(name="ptail", bufs=1, space="PSUM") as pt:
            if static_diag:
                csl = slice(0, OWN)
            else:
                pid = nc.vector.partition_id()
                csl = bass.ts(pid, OWN)
            prod = pers.tile([128, OWN], BF16, tag="prod")
            nc.vector.tensor_tensor(prod[0:64, :], UOwnS[0:64, :],
                                    V2[0:64, csl].bitcast(F32), op=ALU.mult)
            nc.vector.tensor_tensor(prod[64:128, :], UOwnS[64:128, :],
                                    V2[64:128, csl].bitcast(F32),
                                    op=ALU.mult)
            pdg = pt.tile([128, 4], F32, tag="pt")
            for ic in range(4):
                nc.tensor.matmul(pdg[:, ic:ic + 1],
                                 prod[:, ic * 128:(ic + 1) * 128],
                                 bw[:, BONE:BONE + 1], start=True, stop=False)
                nc.tensor.matmul(pdg[:, ic:ic + 1],
                                 bw[:, ZWO + ic * 128:ZWO + (ic + 1) * 128],
                                 bw[:, BOWN:BOWN + 1], start=False, stop=True)
            nc.vector.tensor_copy(vout[:, 16:20], pdg[:])
            nc.sync.dma_start(vout_d[:, 4:20], vout[:, 4:20])
            nc.sync.dma_start(vout_d[:, 0:4], vout[:, 0:4])

    nc.compile()
    return nc


_built = None


def _get_program():
    global _built
    if _built is None:
        _built = _build_program()
    return _built


def make_in_maps(inputs):
    import ml_dtypes
    BF = ml_dtypes.bfloat16
    f = lambda x: np.asarray(x, dtype=np.float32)

    zw = np.concatenate([f(inputs['zw_0']), f(inputs['zw_1'])], axis=0)
    zwT = np.ascontiguousarray(zw.T)                  # [128, 4096]
    # chunk-major layout: block ch = [group0 cols ch*512.. | group1 cols ...]
    zwc = np.empty_like(zwT)
    for ch in range(4):
        zwc[:, ch * 1024:ch * 1024 + 512] = zwT[:, ch * 512:(ch + 1) * 512]
        zwc[:, ch * 1024 + 512:(ch + 1) * 1024] = \
            zwT[:, G + ch * 512:G + (ch + 1) * 512]
    b0 = f(inputs['Ww0_b']) + f(inputs['Wk_b'])
    b1 = f(inputs['Ww1_b']) + f(inputs['Wk_b'])
    wk = f(inputs['Wk_w'])
    uw0 = np.hstack([wk, f(inputs['Ww0_w'])])          # [Czw ; Azw0]
    uw1 = np.hstack([wk, f(inputs['Ww1_w'])])          # [Czw ; Azw1]

    bigw = np.zeros((128, WN), np.float32)
    bigw[:, LW0:LW0 + 64] = f(inputs['lin0_w']).T
    bigw[:, LW1:LW1 + 64] = f(inputs['lin1_w']).T
    bigw[0:64, A1W:A1W + 32] = f(inputs['a0_1w']).T
    bigw[64:128, A1W + 32:A1W + 64] = f(inputs['a1_1w']).T
    bigw[0:32, A2W:A2W + 1] = f(inputs['a0_2w']).T
    bigw[32:64, A2W + 1:A2W + 2] = f(inputs['a1_2w']).T
    bigw[0, SEL:SEL + 64] = 1.0
    bigw[1, SEL + 64:SEL + 128] = 1.0
    bigw[:, BSTK] = b0
    bigw[:, BSTK + 1] = b1
    bigw[:, BONE] = 1.0

    bigr = np.zeros((128, NR), np.float32)
    bigr[:, CU0:CU0 + 128] = uw0
    bigr[:, CU1:CU1 + 128] = uw1
    bigf = np.zeros((128, NF32), np.float32)
    bigf[:, LINB] = np.concatenate([f(inputs['lin0_b']), f(inputs['lin1_b'])])
    bigf[0:64, A1B] = np.concatenate([f(inputs['a0_1b']), f(inputs['a1_1b'])])
    bigf[0:2, A2B] = np.concatenate([f(inputs['a0_2b']), f(inputs['a1_2b'])])

    cT = np.ascontiguousarray(f(inputs['c']).T)        # [64, 4096]

    in_maps = []
    for cid in range(N_CORES):
        g = cid // 4
        mw = bigw.copy()
        mw[:, ZWO:ZWO + OWN] = zwT[:, cid * OWN:(cid + 1) * OWN]
        mw[:, BOWN] = b0 if g == 0 else b1
        mr = bigr.copy()
        mr[:, ZWOF:ZWOF + OWN] = zwT[:, cid * OWN:(cid + 1) * OWN]
        mr[:, CUWO:CUWO + 128] = uw0 if g == 0 else uw1
        in_maps.append({
            'bigw': np.ascontiguousarray(mw.astype(BF)),
            'zwc': np.ascontiguousarray(zwc.astype(BF)),
            'bigr': np.ascontiguousarray(mr),
            'bigf': bigf,
            'cT': cT,
        })
    return in_maps


def kernel(**inputs):
    nc = _get_program()
    in_maps = make_in_maps(inputs)
    res = run_bass_kernel_spmd(nc, in_maps, list(range(N_CORES)))
    tot = 0.0
    for r in res.results:
        v = np.asarray(r['vout'], dtype=np.float64)
        v[:, 4] = v[:, 12] + v[:, 13]       # split chunks (g1, ic0/ic1)
        v[:, 5] = v[:, 14] + v[:, 15]
        se = v[:, 0:4] + v[:, 4:8]          # [128, 4]: sum over both groups
        dg = v[:, 8:12]
        tot += np.sum(dg - SHIFT - np.log(se))
    return np.array(-(tot / B), dtype=np.float32)


# revision 38
# speedup vs baseline: 1.1214x; 1.1041x over previous
"""Trainium2 Bass kernel for nn_CPCModel (CPC-style NCE loss).

Strategy (8 NeuronCores, full inputs on every core, no collectives):

The reference's leave-one-out softmax pooling collapses algebraically:
    pooled[i] = (T - e_i * zt_i) / (S - e_i),  e = exp(s), S = sum(e), T = sum(e_j zt_j)
so the [B,B] pooling matrix is never materialized.  The loss needs only
    nce = -mean_i( total[i,i] - logsumexp_j total[i,j] )
with  total[i, j in group g] = Azw_g[i]·pooled_g[j] + Czw[i]·c[j] + delta_g[i]
where Azw_g = zw @ Ww_g, Czw = zw @ Wk_w, delta_g = zw @ (Ww_g_b + Wk_b).

Each core redundantly computes the cheap pooling prep for all 4096 rows
(both groups stacked on the 128 partitions) and its own 512 rows of the
[4096,4096] total matrix + row-wise sum(exp(total - 44)) via 8
[128,2048] exp chunks on the scalar engine with accum_out.  The loop
rhs V2 = [cT ; pooled0|pooled1] is assembled by three DMAs (no compute)
and doubles as the diagonal's source through one partition_id-dynamic
column slice.  Raw per-row exp sums and raw diagonal values ship to the
host, which finishes with log() in float64.

Scheduling: weights arrive in a small first DMA and zw^T in two
chunk-major halves so the zt->h->s chain starts ~4us in; exp-of-s is
chunked so the fused ztw/T tensor_tensor_reduce starts before the last
tanh; U/bias matmuls and paced PE probes keep the tensor engine out of
its low p-state before the main loop.  fp32r matmuls keep K=128
throughout (K=64 fp32r dies at runtime).  One activation-table set
(exp_and_others: relu/tanh/exp/copy) serves the whole kernel; the final
log runs on the host.
"""

import numpy as np

import concourse.bacc as bacc
import concourse.bass as bass
import concourse.mybir as mybir
import concourse.tile as tile
from concourse.bass_utils import run_bass_kernel_spmd

N_CORES = 8
B = 4096
OWN = B // N_CORES            # 512 rows of `total` per core
G = 2048                      # group size
F32 = mybir.dt.float32
F32R = mybir.dt.float32r
BF16 = mybir.dt.bfloat16
AF = mybir.ActivationFunctionType
ALU = mybir.AluOpType
SHIFT = 44.0

# bf16 weights bundle column offsets
ZWO = 0           # zwoTb [128, 512]
LW0 = 512         # lin0_w.T [128, 64]
LW1 = 576         # lin1_w.T [128, 64]
A1W = 640         # blockdiag(a0_1w.T, a1_1w.T) [128, 64]
A2W = 704         # blockdiag(a0_2w.T, a1_2w.T) [64, 2]
SEL = 706         # sel2 [2, 128]
BSTK = 834        # [b0 | b1] [128, 2]
BOWN = 836        # b_{group(core)} [128, 1]
BONE = 837        # ones [128, 1]
WN = 838

# f32r bundle column offsets
ZWOF = 0          # zwoT [128, 512]
CU0 = 512         # hstack(Wk_w, Ww0_w) [128, 128]
CU1 = 640         # hstack(Wk_w, Ww1_w)
CUWO = 768        # per-core hstack(Wk_w, Ww_g) [128, 128] (diag)
NR = 896
# f32 bundle column offsets
LINB = 0          # [lin0_b ; lin1_b] [128, 1]
A1B = 1           # [a0_1b ; a1_1b] [64, 1]
A2B = 2           # [a0_2b ; a1_2b] [2, 1]
NF32 = 3


def _build_program(static_diag=False):
    nc = bacc.Bacc(
        "TRN2",
        target_bir_lowering=False,
        debug=False,
        num_devices=N_CORES,
    )

    bigw_d = nc.dram_tensor("bigw", [128, WN], BF16, kind="ExternalInput").ap()
    zwc_d = nc.dram_tensor("zwc", [128, B], BF16, kind="ExternalInput").ap()
    bigr_d = nc.dram_tensor("bigr", [128, NR], F32R, kind="ExternalInput").ap()
    bigf_d = nc.dram_tensor("bigf", [128, NF32], F32, kind="ExternalInput").ap()
    cT_d = nc.dram_tensor("cT", [64, B], F32R, kind="ExternalInput").ap()
    vout_d = nc.dram_tensor("vout", [128, 20], F32, kind="ExternalOutput").ap()

    from contextlib import ExitStack
    with tile.TileContext(nc) as tc, ExitStack() as ctx:
        pers = ctx.enter_context(tc.tile_pool(name="pers", bufs=1))
        scr = ctx.enter_context(tc.tile_pool(name="scr", bufs=2))

        # DMA order = need order: weights, zw halves, f32r/f32 bundles, cT
        bw = pers.tile([128, WN], BF16, tag="bw", name="bw")
        nc.sync.dma_start(bw[:], bigw_d[:])
        zwc = pers.tile([128, B], BF16, tag="zwc", name="zwc")
        nc.sync.dma_start(zwc[:, 0:1024], zwc_d[:, 0:1024])
        bf = pers.tile([128, NF32], F32, tag="bf", name="bf")
        nc.sync.dma_start(bf[:], bigf_d[:])
        for p in range(1, 4):
            nc.sync.dma_start(zwc[:, p * 1024:(p + 1) * 1024],
                              zwc_d[:, p * 1024:(p + 1) * 1024])
        br = pers.tile([128, NR], F32R, tag="br", name="br")
        nc.sync.dma_start(br[:], bigr_d[:])
        # V2 = [cT ; pooled0|pooled1]: loop rhs AND diagonal source
        V2 = pers.tile([128, B], F32R, tag="V2", name="V2")
        nc.sync.dma_start(V2[0:64, :], cT_d[:])

        zwoT = br[:, ZWOF:ZWOF + 512]
        sel2 = bw[0:2, SEL:SEL + 128]
        linb2 = bf[:, LINB:LINB + 1]
        a1b2 = bf[0:64, A1B:A1B + 1]
        a2b2 = bf[0:2, A2B:A2B + 1]

        ztT2 = pers.tile([128, G], BF16, tag="ztT2")
        hT2 = pers.tile([64, G], BF16, tag="hT2")
        eT2 = pers.tile([2, G], BF16, tag="eT2")
        d2 = pers.tile([2, G], BF16, tag="d2")
        bT2 = pers.tile([2, G], BF16, tag="bT2")
        ztwT2 = pers.tile([128, G], F32, tag="ztwT2")
        pooled2 = pers.tile([128, G], F32R, tag="pooled2")
        Sacc = pers.tile([2, 4], F32, tag="Sacc")
        Tacc = pers.tile([128, 4], F32, tag="Tacc")
        T2 = pers.tile([128, 1], F32, tag="T2")
        S2 = pers.tile([2, 1], F32, tag="S2")
        biasS = pers.tile([128, 8], F32, tag="biasS")
        U0 = pers.tile([128, OWN], F32R, tag="U0")
        U1 = pers.tile([128, OWN], F32R, tag="U1")
        UOwnS = pers.tile([128, OWN], F32, tag="UOwnS")
        vout = pers.tile([128, 20], F32, tag="vout")

        actwarm = pers.tile([2, 1], BF16, tag="actwarm")
        junkw = pers.tile([128, 512], BF16, tag="junkw")
        nc.gpsimd.memset(junkw[:], 1.0)

        with tc.tile_pool(name="psA", bufs=6, space="PSUM") as psA, \
             tc.tile_pool(name="psB", bufs=1, space="PSUM") as psB:
            psU = psA
            # trigger the one-time activation table load before tanh needs it
            nc.scalar.activation(actwarm[:], bw[0:2, 0:1], AF.Tanh)
            # PE warmups on a memset tile: ramp the p-state from ~1us, long
            # before the first DMA lands, and hand off hot to the zt chain
            for _ in range(9):
                pw0 = psA.tile([128, 512], F32, tag="ps")
                nc.tensor.matmul(pw0[:], junkw[:, 0:128], junkw[:],
                                 start=True, stop=True)

            # ---- zt -> h -> s -> e chain, issued breadth-first so each
            # engine's in-order queue never blocks the next chunk ----
            SL = [slice(ch * 512, (ch + 1) * 512) for ch in range(4)]
            pzs = []
            for ch in range(4):
                pz = psA.tile([128, 512], F32, tag="ps")
                nc.tensor.matmul(pz[0:64, :], bw[:, LW0:LW0 + 64],
                                 zwc[:, ch * 1024:ch * 1024 + 512],
                                 start=True, stop=True)
                nc.tensor.matmul(pz[64:128, :], bw[:, LW1:LW1 + 64],
                                 zwc[:, ch * 1024 + 512:(ch + 1) * 1024],
                                 start=True, stop=True)
                pzs.append(pz)
            for ch in range(4):
                # relu(x + bias): split across Act and DVE
                if ch % 2 == 0:
                    nc.scalar.activation(ztT2[:, SL[ch]], pzs[ch][:], AF.Relu,
                                         bias=linb2)
                else:
                    nc.vector.tensor_scalar(ztT2[:, SL[ch]], pzs[ch][:], linb2,
                                            0.0, op0=ALU.add, op1=ALU.max)
            phs = []
            for ch in range(4):
                ph = psA.tile([128, 512], F32, tag="ps")
                nc.tensor.matmul(ph[0:64, :], bw[:, A1W:A1W + 64],
                                 ztT2[:, SL[ch]], start=True, stop=True)
                phs.append(ph)
            pss = []
            for ch in range(4):
                ps_ = psA.tile([128, 512], F32, tag="ps")
                pss.append(ps_)
            for ch in range(4):
                nc.scalar.activation(hT2[:, SL[ch]], phs[ch][0:64, :], AF.Tanh,
                                     bias=a1b2)
                nc.tensor.matmul(pss[ch][0:2, :], bw[0:64, A2W:A2W + 2],
                                 hT2[:, SL[ch]], start=True, stop=True)
                nc.scalar.activation(eT2[:, SL[ch]], pss[ch][0:2, :], AF.Exp,
                                     bias=a2b2, accum_out=Sacc[:, ch:ch + 1])

            # ---- e-broadcast, ztw on DVE, T-partials on Act (accum) ----
            pebs = []
            for ch in range(4):
                peb = psA.tile([128, 512], F32, tag="ps")
                nc.tensor.matmul(peb[:], sel2, eT2[:, SL[ch]],
                                 start=True, stop=True)
                pebs.append(peb)
            for ch in range(4):
                nc.vector.tensor_tensor(ztwT2[:, SL[ch]], ztT2[:, SL[ch]],
                                        pebs[ch][:], op=ALU.mult)
                tjunk = scr.tile([128, 512], BF16, tag="tj")
                nc.scalar.activation(tjunk[:], ztwT2[:, SL[ch]], AF.Copy,
                                     accum_out=Tacc[:, ch:ch + 1])

            # ---- beta = 1/(e - S) on DVE ----
            nc.vector.reduce_sum(S2[:], Sacc[:], axis=mybir.AxisListType.X)
            nc.vector.tensor_scalar(d2[:], eT2[:], S2[:], None,
                                    op0=ALU.subtract)

            # ---- U tiles + delta biases (PE slack while beta computes) ----
            upus = []
            for uoff in (CU0, CU1):
                pu = psA.tile([128, 512], F32, tag="ps")
                nc.tensor.matmul(pu[:], br[:, uoff:uoff + 128],
                                 zwoT, start=True, stop=True)
                upus.append(pu)
            puo = psA.tile([128, 512], F32, tag="ps")
            nc.tensor.matmul(puo[:], br[:, CUWO:CUWO + 128],
                             zwoT, start=True, stop=True)
            nc.scalar.copy(UOwnS[:], puo[:])
            pbias = psB.tile([128, 8], F32, tag="pb")
            for ic in range(4):
                nc.tensor.matmul(pbias[:, ic * 2:ic * 2 + 2],
                                 bw[:, ZWO + ic * 128:ZWO + (ic + 1) * 128],
                                 bw[:, BSTK:BSTK + 2], start=True, stop=True)
            nc.vector.tensor_scalar(biasS[:], pbias[:], -SHIFT, None,
                                    op0=ALU.add)

            # ---- beta chunks, T2, pooled combines: interleaved on DVE.
            # Group-1 combines write straight into V2 (partition-aligned), so
            # the loop's first exp chunks start without any DMA hop; group-0
            # combines land in pooled2 and shift to V2 via DMA during the
            # group-1 half of the loop.
            def recip(ch):
                with nc.allow_low_precision(reason="beta in bf16 for PE bcast"):
                    nc.vector.reciprocal(bT2[:, SL[ch]], d2[:, SL[ch]])

            def bcast(ch):
                pbb = psA.tile([128, 512], F32, tag="ps", name=f"pbb{ch}")
                nc.tensor.matmul(pbb[:], sel2, bT2[:, SL[ch]],
                                 start=True, stop=True)
                return pbb

            def combine(ch, pbb):
                nc.vector.scalar_tensor_tensor(
                    out=pooled2[:, SL[ch]], in0=ztwT2[:, SL[ch]], scalar=T2[:],
                    in1=pbb[:], op0=ALU.subtract, op1=ALU.mult)

            recip(0)
            pbb0 = bcast(0)
            recip(1)
            pbb1 = bcast(1)
            nc.vector.reduce_sum(T2[:], Tacc[:], axis=mybir.AxisListType.X)
            combine(0, pbb0)
            # group-1 V2 pieces stream per-chunk: the loop runs group 1 first
            nc.sync.dma_start(V2[64:128, G:G + 512], pooled2[64:128, 0:512])
            recip(2)
            pbb2 = bcast(2)
            combine(1, pbb1)
            nc.sync.dma_start(V2[64:128, G + 512:G + 1024],
                              pooled2[64:128, 512:1024])
            recip(3)
            pbb3 = bcast(3)
            combine(2, pbb2)
            nc.sync.dma_start(V2[64:128, G + 1024:G + 1536],
                              pooled2[64:128, 1024:1536])
            combine(3, pbb3)
            nc.sync.dma_start(V2[64:128, G + 1536:B], pooled2[64:128, 1536:G])
            nc.sync.dma_start(V2[64:128, 0:1024], pooled2[0:64, 0:1024])
            nc.sync.dma_start(V2[64:128, 1024:G], pooled2[0:64, 1024:G])
            # paced junk matmuls: keep the PE out of its low p-state between
            # the prep matmuls and the main loop
            for ch in range(4):
                pw = psA.tile([128, 512], F32, tag="ps")
                nc.tensor.matmul(pw[:], br[:, CU0:CU0 + 128],
                                 pooled2[:, SL[ch]], start=True, stop=True)

            # U copies on Act: needed only once the loop starts
            nc.scalar.copy(U1[:], upus[1][:])
            nc.scalar.copy(U0[:], upus[0][:])

        # ---- main loop over the [512 own rows, 4096 cols] of `total`:
        # mostly 2048-col exp chunks; the first two group-1 chunks split in
        # 1024-col halves so exp work starts as soon as the first half of
        # pooled1 lands in V2 (the rest of the tail still streaming) ----
        with tc.tile_pool(name="pbig", bufs=2, space="PSUM") as pbig:
            def loop_chunk(g, ic, h, width, cc):
                usl = slice(ic * 128, (ic + 1) * 128)
                Ug = U0 if g == 0 else U1
                base = g * G + h * 1024
                pm = pbig.tile([128, width], F32, tag="pb",
                               padded_shape=[128, G])
                for q in range(width // 512):
                    qs = slice(q * 512, (q + 1) * 512)
                    nc.tensor.matmul(pm[:, qs], Ug[:, usl],
                                     V2[:, base + q * 512:
                                        base + (q + 1) * 512],
                                     start=True, stop=True)
                es = scr.tile([128, width], BF16, tag="es",
                              padded_shape=[128, G])
                nc.scalar.activation(es[:], pm[:], AF.Exp,
                                     bias=biasS[:, 2 * ic + g:2 * ic + g + 1],
                                     accum_out=vout[:, cc:cc + 1])

            loop_chunk(1, 0, 0, 1024, 12)
            loop_chunk(1, 0, 1, 1024, 13)
            loop_chunk(1, 1, 0, 1024, 14)
            loop_chunk(1, 1, 1, 1024, 15)
            for g, ic in [(1, 2), (1, 3), (0, 0), (0, 1), (0, 2), (0, 3)]:
                loop_chunk(g, ic, 0, G, g * 4 + ic)

        # ---- diagonal: diag[i] = UOwn[:,i]·V2[:,own(i)] + delta ----
        with tc.tile_pool(name="ptail", bufs=1, space="PSUM") as pt:
            if static_diag:
                csl = slice(0, OWN)
            else:
                pid = nc.vector.partition_id()
                csl = bass.ts(pid, OWN)
            prod = pers.tile([128, OWN], BF16, tag="prod")
            nc.vector.tensor_tensor(prod[0:64, :], UOwnS[0:64, :],
                                    V2[0:64, csl].bitcast(F32), op=ALU.mult)
            nc.vector.tensor_tensor(prod[64:128, :], UOwnS[64:128, :],
                                    V2[64:128, csl].bitcast(F32),
                                    op=ALU.mult)
            pdg = pt.tile([128, 4], F32, tag="pt")
            for ic in range(4):
                nc.tensor.matmul(pdg[:, ic:ic + 1],
                                 prod[:, ic * 128:(ic + 1) * 128],
                                 bw[:, BONE:BONE + 1], start=True, stop=False)
                nc.tensor.matmul(pdg[:, ic:ic + 1],
                                 bw[:, ZWO + ic * 128:ZWO + (ic + 1) * 128],
                                 bw[:, BOWN:BOWN + 1], start=False, stop=True)
            nc.vector.tensor_copy(vout[:, 16:20], pdg[:])
            nc.sync.dma_start(vout_d[:, 4:20], vout[:, 4:20])
            nc.sync.dma_start(vout_d[:, 0:4], vout[:, 0:4])

    nc.compile()
    return nc


_built = None


def _get_program():
    global _built
    if _built is None:
        _built = _build_program()
    return _built


def make_in_maps(inputs):
    import ml_dtypes
    BF = ml_dtypes.bfloat16
    f = lambda x: np.asarray(x, dtype=np.float32)

    zw = np.concatenate([f(inputs['zw_0']), f(inputs['zw_1'])], axis=0)
    zwT = np.ascontiguousarray(zw.T)                  # [128, 4096]
    # chunk-major layout: block ch = [group0 cols ch*512.. | group1 cols ...]
    zwc = np.empty_like(zwT)
    for ch in range(4):
        zwc[:, ch * 1024:ch * 1024 + 512] = zwT[:, ch * 512:(ch + 1) * 512]
        zwc[:, ch * 1024 + 512:(ch + 1) * 1024] = \
            zwT[:, G + ch * 512:G + (ch + 1) * 512]
    b0 = f(inputs['Ww0_b']) + f(inputs['Wk_b'])
    b1 = f(inputs['Ww1_b']) + f(inputs['Wk_b'])
    wk = f(inputs['Wk_w'])
    uw0 = np.hstack([wk, f(inputs['Ww0_w'])])          # [Czw ; Azw0]
    uw1 = np.hstack([wk, f(inputs['Ww1_w'])])          # [Czw ; Azw1]

    bigw = np.zeros((128, WN), np.float32)
    bigw[:, LW0:LW0 + 64] = f(inputs['lin0_w']).T
    bigw[:, LW1:LW1 + 64] = f(inputs['lin1_w']).T
    bigw[0:64, A1W:A1W + 32] = f(inputs['a0_1w']).T
    bigw[64:128, A1W + 32:A1W + 64] = f(inputs['a1_1w']).T
    bigw[0:32, A2W:A2W + 1] = f(inputs['a0_2w']).T
    bigw[32:64, A2W + 1:A2W + 2] = f(inputs['a1_2w']).T
    bigw[0, SEL:SEL + 64] = 1.0
    bigw[1, SEL + 64:SEL + 128] = 1.0
    bigw[:, BSTK] = b0
    bigw[:, BSTK + 1] = b1
    bigw[:, BONE] = 1.0

    bigr = np.zeros((128, NR), np.float32)
    bigr[:, CU0:CU0 + 128] = uw0
    bigr[:, CU1:CU1 + 128] = uw1
    bigf = np.zeros((128, NF32), np.float32)
    bigf[:, LINB] = np.concatenate([f(inputs['lin0_b']), f(inputs['lin1_b'])])
    bigf[0:64, A1B] = np.concatenate([f(inputs['a0_1b']), f(inputs['a1_1b'])])
    bigf[0:2, A2B] = np.concatenate([f(inputs['a0_2b']), f(inputs['a1_2b'])])

    cT = np.ascontiguousarray(f(inputs['c']).T)        # [64, 4096]

    in_maps = []
    for cid in range(N_CORES):
        g = cid // 4
        mw = bigw.copy()
        mw[:, ZWO:ZWO + OWN] = zwT[:, cid * OWN:(cid + 1) * OWN]
        mw[:, BOWN] = b0 if g == 0 else b1
        mr = bigr.copy()
        mr[:, ZWOF:ZWOF + OWN] = zwT[:, cid * OWN:(cid + 1) * OWN]
        mr[:, CUWO:CUWO + 128] = uw0 if g == 0 else uw1
        in_maps.append({
            'bigw': np.ascontiguousarray(mw.astype(BF)),
            'zwc': np.ascontiguousarray(zwc.astype(BF)),
            'bigr': np.ascontiguousarray(mr),
            'bigf': bigf,
            'cT': cT,
        })
    return in_maps


def kernel(**inputs):
    nc = _get_program()
    in_maps = make_in_maps(inputs)
    res = run_bass_kernel_spmd(nc, in_maps, list(range(N_CORES)))
    tot = 0.0
    for r in res.results:
        v = np.asarray(r['vout'], dtype=np.float64)
        v[:, 4] = v[:, 12] + v[:, 13]       # split chunks (g1, ic0/ic1)
        v[:, 5] = v[:, 14] + v[:, 15]
        se = v[:, 0:4] + v[:, 4:8]          # [128, 4]: sum over both groups
        dg = v[:, 8:12]
        tot += np.sum(dg - SHIFT - np.log(se))
    return np.array(-(tot / B), dtype=np.float32)
